# revision 25
# baseline (speedup 1.0000x reference)
"""Causal multi-head attention kernel for 8 Trainium2 NeuronCores.

Problem: x(4,2048,512) -> qkv proj -> 8-head causal attention -> out proj.
Sharding: core c handles batch b=c//2, heads 4*(c%2)..4*(c%2)+3.
Each core returns a partial (2048,512) output (its 4 heads' contribution
through w_out); host sums the two cores of each batch and adds b_out.

The attention inner loop is ACT(exp)-bound (~1 elem/cycle/lane), so the
kernel software-pipelines every other stage under it:
  - input DMA split into consumption-ordered chunks (weights -> xT sc0 ->
    remaining xT) so the QKV projection starts ~5us in;
  - Q/K projections (per 512-seq chunk), V-aug projection (per 128-seq
    chunk) and the output projection (per 128-row chunk) are emitted as
    "filler" PE work interleaved into the attention kk-loop;
  - causal diag masking is a bf16 DVE multiply on the exp'd tile (PE does
    no mask matmuls), Q/K biases fold into the DVE PSUM->SBUF evacuation
    (per-partition tensor_scalar add), V bias + denominator ones-column
    fold into a host-precomputed (128,260) tile added during evacuation;
  - output chunks DMA to HBM as soon as each 128-row slice is projected.

Per-core device algorithm (bf16 matmuls, fp32 psum/softmax):
  P1  QT/KT stored (128=2heads, 2048) per pair; vaug holds [V|1] per
      128-seq chunk so the PV matmul also produces softmax denominators.
  P2  per head-pair: S_T = K Q^T (k on partitions, q free, two heads
      packed into PE row-groups 0/64 -> concurrent), exp via ACT
      (scale=1/8 folded, no max subtraction -- scores are O(7)), diag
      mask via DVE multiply, then out'/denom = [V|1]^T @ P~^T in PSUM.
  P3  denominators DMA-gathered to (16,64), one DVE reciprocal, DMA back
      to a row, partition-broadcast via a ones(1,64) matmul, normalize,
      project, stream out per 128-row chunk.
"""

import os
import sys

import numpy as np

if "/opt/trn_rl_repo" not in sys.path:
    sys.path.insert(0, "/opt/trn_rl_repo")

import ml_dtypes

import concourse.bass as bass
import concourse.mybir as mybir
import concourse.tile as tile
from concourse import bacc
from concourse.bass_utils import run_bass_kernel_spmd

F32 = mybir.dt.float32
BF16 = mybir.dt.bfloat16
AF = mybir.ActivationFunctionType

S = 2048
D = 512
HD = 64
HPC = 4          # heads per core
NCORES = 8
SCALE = 0.125    # 1/sqrt(64)
VW = HD + 1      # 65: V plus ones column
VWS = HPC * VW   # 260

# column offsets inside the packed bf16 (128, FTOT) input, ordered by
# consumption so the input DMA can stream in 3 contiguous chunks:
# [everything the prefix needs][xT sc1-3][wo]
OFF_BQK = 0                     # (128,4): bq_p0, bq_p1, bk_p0, bk_p1 cols
OFF_ONES64 = 4                  # row 0: ones(64)
OFF_TRI = 68                    # (128,256): tri|tri, tri[k,x]=1 if k<=x
OFF_VB = OFF_TRI + 256          # (128,260): [bv|1] per head, replicated
OFF_WQ = OFF_VB + VWS           # 4 tiles of (128, 256)
OFF_WK = OFF_WQ + 4 * 256
OFF_WVA = OFF_WK + 4 * 256      # 4 tiles of (128, 260)
OFF_XT = OFF_WVA + 4 * VWS      # 4 sc-chunks of (4 dc x (128, 512))
OFF_WO = OFF_XT + 4 * S         # 2 tiles of (128, 512)
FTOT = OFF_WO + 2 * D


def build_nc():
    nc = bacc.Bacc("TRN2", target_bir_lowering=False, debug=False)

    wpack = nc.dram_tensor("wpack", [128, FTOT], BF16,
                           kind="ExternalInput").ap()
    out = nc.dram_tensor("out", [S, D], F32, kind="ExternalOutput").ap()

    with tile.TileContext(nc) as tc:
        _build_kernel(tc, wpack, out)
    nc.compile()
    return nc


def _build_kernel(tc, wpack, out):
    nc = tc.nc
    from contextlib import ExitStack

    ctx = ExitStack()
    with ctx:
        pers = ctx.enter_context(tc.tile_pool(name="pers", bufs=1))
        spsum = ctx.enter_context(
            tc.tile_pool(name="spsum", bufs=2, space="PSUM"))   # scores
        opsum = ctx.enter_context(
            tc.tile_pool(name="opsum", bufs=1, space="PSUM"))   # PV accum
        upsum = ctx.enter_context(
            tc.tile_pool(name="upsum", bufs=2, space="PSUM"))   # p1/p3/bcast
        ptp = ctx.enter_context(tc.tile_pool(name="ptp", bufs=4))
        otexp = ctx.enter_context(tc.tile_pool(name="otexp", bufs=4))
        outp = ctx.enter_context(tc.tile_pool(name="outp", bufs=3))
        dnp = ctx.enter_context(tc.tile_pool(name="dnp", bufs=2))

        # ---------- input DMA, split in consumption order ----------
        wr = pers.tile([128, FTOT], BF16, tag="wr", name="wr")
        dma_splits = [
            (0, OFF_XT + S),                   # header + weights + xT sc0
            (OFF_XT + S, OFF_XT + 4 * S),      # xT sc1-3
            (OFF_WO, FTOT),                    # wo (first needed by P3)
        ]
        for a, b in dma_splits:
            nc.sync.dma_start(wr[:, a:b], wpack[:, a:b])

        def xt_sb(dc, sc):
            base = OFF_XT + S * sc + 512 * dc
            return wr[:, base:base + 512]

        def xt_128(st):
            sc, r = st // 4, st % 4
            base = OFF_XT + S * sc
            return [wr[:, base + 512 * dc + 128 * r:
                       base + 512 * dc + 128 * r + 128] for dc in range(4)]

        wq_sb = [wr[:, OFF_WQ + 256 * dc:OFF_WQ + 256 * (dc + 1)]
                 for dc in range(4)]
        wk_sb = [wr[:, OFF_WK + 256 * dc:OFF_WK + 256 * (dc + 1)]
                 for dc in range(4)]
        wva_sb = [wr[:, OFF_WVA + VWS * dc:OFF_WVA + VWS * (dc + 1)]
                  for dc in range(4)]
        wo_sb = [wr[:, OFF_WO + D * p:OFF_WO + D * (p + 1)]
                 for p in range(2)]
        vb_sb = wr[:, OFF_VB:OFF_VB + VWS]
        # per-partition bias columns for tensor_scalar must be fp32
        bcols = pers.tile([128, 4], F32, tag="bcols", name="bcols")
        nc.vector.tensor_copy(bcols[:], wr[:, OFF_BQK:OFF_BQK + 4])
        bq_col = [bcols[:, p:p + 1] for p in range(2)]
        bk_col = [bcols[:, 2 + p:3 + p] for p in range(2)]
        ones64 = wr[0:1, OFF_ONES64:OFF_ONES64 + 64]
        tri2 = wr[:, OFF_TRI:OFF_TRI + 256].rearrange(
            "p (b c) -> p b c", c=128)

        # ---------- persistent state ----------
        QT, KT, OTN = [], [], []
        for p in range(2):
            QT.append(pers.tile([128, S], BF16, tag=f"QT{p}", name=f"QT{p}"))
            KT.append(pers.tile([128, S], BF16, tag=f"KT{p}", name=f"KT{p}"))
            OTN.append(pers.tile([128, S], BF16, tag=f"OTN{p}",
                                 name=f"OTN{p}"))
        vaug = pers.tile([128, 16 * VWS], BF16, tag="vaug", name="vaug")

        # memset the scores psum buffers once (safety for first-use lanes)
        # and run a throwaway exp on one so the ACT table load (~2.7us)
        # happens during the input-DMA wait instead of at the first score
        warm = pers.tile([1, 8], BF16, tag="warm", name="warm")
        for i in range(2):
            ps_init = spsum.tile([128, 1024], F32, tag="ps_s", name="ps_i")
            nc.vector.memset(ps_init[:], 0.0)
            if i == 0:
                nc.scalar.activation(warm[:], ps_init[0:1, 0:8], AF.Exp,
                                     scale=SCALE)

        # ---------- work units ----------
        # p1 units are generators yielding after every matmul, so the main
        # loop can interleave filler PE work at single-matmul granularity
        # (a whole 4-matmul unit in one burst delays the next QK enough to
        # starve the exp stream)
        def p1a_unit(p, which, sc):
            """Project one 512-seq chunk of Q or K for pair p."""
            w_sb = wq_sb if which == "q" else wk_sb
            b_col = bq_col[p] if which == "q" else bk_col[p]
            dst = QT[p] if which == "q" else KT[p]
            ps = upsum.tile([128, 512], F32, tag="util", name="p1ps")
            for dc in range(4):
                nc.tensor.matmul(
                    ps[:],
                    w_sb[dc][:, 128 * p:128 * (p + 1)],
                    xt_sb(dc, sc),
                    start=(dc == 0), stop=(dc == 3))
                yield
            nc.vector.tensor_scalar_add(
                dst[:, 512 * sc:512 * (sc + 1)], ps[:], b_col)
            yield

        def p1b_unit(st):
            """Project one 128-seq chunk of [V|1] (all 4 heads)."""
            xbl = xt_128(st)
            ps = upsum.tile([128, 512], F32, tag="util", name="p1vps")
            for dc in range(4):
                nc.tensor.matmul(
                    ps[:, 0:VWS],
                    xbl[dc],
                    wva_sb[dc][:],
                    start=(dc == 0), stop=(dc == 3))
                yield
            nc.vector.tensor_add(vaug[:, VWS * st:VWS * (st + 1)],
                                 ps[:, 0:VWS], vb_sb)
            yield

        def run_unit(gen):
            for _ in gen:
                pass

        def p3_unit(t):
            """Project + stream out one 128-row output chunk."""
            ps = upsum.tile([128, 512], F32, tag="util", name="p3ps")
            for p in range(2):
                nc.tensor.matmul(
                    ps[:],
                    OTN[p][:, 128 * t:128 * (t + 1)],
                    wo_sb[p][:],
                    start=(p == 0), stop=(p == 1))
            osb = outp.tile([128, 512], F32, tag="osb", name="osb")
            nc.vector.tensor_copy(osb[:], ps[:])
            nc.sync.dma_start(out[128 * t:128 * (t + 1), :], osb[:])

        # ---------- prefix: minimum P1 to start pair 0's attention ----------
        for st in range(4):
            run_unit(p1b_unit(st))
        run_unit(p1a_unit(0, "q", 0))
        run_unit(p1a_unit(0, "k", 0))

        # ---------- main attention loop, fillers interleaved ----------
        for qq in range(4):
            units = []
            if qq == 0:
                # pair 1's sc0 projections must land before its kk loop
                units.append(p1a_unit(1, "q", 0))
                units.append(p1a_unit(1, "k", 0))
            if qq < 3:
                for p in range(2):
                    units.append(p1a_unit(p, "q", qq + 1))
                    units.append(p1a_unit(p, "k", qq + 1))
                for st in range(4 * qq + 4, 4 * qq + 8):
                    units.append(p1b_unit(st))
            # one flat stream of single-matmul filler steps for this qq.
            # state[0] is the in-flight unit: it must be drained to a unit
            # boundary before anything else allocates from the util ring
            # (an open accumulation group + 2 more allocs would wrap the
            # 2-buffer ring and clobber the accumulating psum tile)
            state = [None]
            nstep0 = len(units) * 5
            nstep = 0

            def step_filler():
                while True:
                    if state[0] is None:
                        if not units:
                            return False
                        state[0] = units.pop(0)
                    try:
                        next(state[0])
                        return True
                    except StopIteration:
                        state[0] = None

            def drain_unit_boundary():
                if state[0] is not None:
                    for _ in state[0]:
                        pass
                    state[0] = None

            nkk = 4 * qq + 4
            niter = 2 * nkk
            it = 0
            for p in range(2):
                ps_oo = []
                for sub in range(2):
                    t = opsum.tile([VW, 512], F32, tag=f"ps_o{sub}",
                                   name=f"ps_o{sub}")
                    ps_oo.append(t)
                for kk in range(nkk):
                    diag = (kk >= 4 * qq)
                    so = 128 * (kk - 4 * qq) if diag else 0
                    ps_s = spsum.tile([128, 1024], F32, tag="ps_s",
                                      name="ps_s")
                    for sub in range(2):
                        qrows = slice(64 * sub, 64 * sub + 64)
                        nc.tensor.matmul(
                            ps_s[:, 512 * sub + so:512 * (sub + 1)],
                            KT[p][qrows, 128 * kk:128 * (kk + 1)],
                            QT[p][qrows, 512 * qq + so:512 * (qq + 1)],
                            start=True, stop=True)
                    pt = ptp.tile([128, 1024], BF16, tag="pt", name="pt")
                    if so == 0:
                        nc.scalar.activation(pt[:], ps_s[:], AF.Exp,
                                             scale=SCALE)
                    else:
                        pss3 = ps_s[:].rearrange("p (b c) -> p b c", c=512)
                        pt3 = pt[:].rearrange("p (b c) -> p b c", c=512)
                        nc.scalar.activation(pt3[:, :, so:], pss3[:, :, so:],
                                             AF.Exp, scale=SCALE)
                    if diag:
                        # zero the strictly-upper (k>q) part of the 128-wide
                        # diag block on DVE: one bf16 multiply, both heads
                        ptd = pt[:].rearrange(
                            "p (b c) -> p b c", c=512)[:, :, so:so + 128]
                        nc.vector.tensor_mul(ptd, ptd, tri2)
                    for sub in range(2):
                        h = 2 * p + sub
                        nc.tensor.matmul(
                            ps_oo[sub][:, so:512],
                            vaug[:, VWS * kk + VW * h:VWS * kk + VW * h + VW],
                            pt[:, 512 * sub + so:512 * (sub + 1)],
                            start=(kk == 0), stop=(kk == nkk - 1))
                    it += 1
                    # interleave filler PE work under the ACT-bound loop:
                    # after iteration `it`, nstep0*it/niter steps are due
                    while nstep < nstep0 * it // niter:
                        if not step_filler():
                            nstep = nstep0
                            break
                        nstep += 1
                # evacuate PV accum + gather denominators for this (p, qq).
                # the last qq's gathers ride the scalar HWDGE ring: ACT is
                # idle by then and the sync ring is busy streaming output
                drain_unit_boundary()
                dma_eng = nc.scalar if qq == 3 else nc.sync
                dq = dnp.tile([16, 64], F32, tag="dq", name="dq")
                otex = []
                for sub in range(2):
                    ot = otexp.tile([VW, 512], F32, tag="otex", name="otex")
                    nc.vector.tensor_copy(ot[:], ps_oo[sub][:])
                    dma_eng.dma_start(dq[8 * sub:8 * sub + 8, :],
                                      ot[64:65, :])
                    otex.append(ot)
                rq = dnp.tile([16, 64], BF16, tag="rq", name="rq")
                with nc.allow_low_precision(reason="bf16 softmax recip"):
                    nc.vector.reciprocal(rq[:], dq[:])
                rrow = dnp.tile([1, 1024], BF16, tag="rrow", name="rrow")
                dma_eng.dma_start(rrow[:], rq[:])
                for sub in range(2):
                    ps_b = upsum.tile([64, 512], F32, tag="util",
                                      name="ps_b")
                    nc.tensor.matmul(
                        ps_b[:],
                        ones64,
                        rrow[0:1, 512 * sub:512 * (sub + 1)],
                        start=True, stop=True)
                    nc.vector.tensor_mul(
                        OTN[p][64 * sub:64 * sub + 64,
                               512 * qq:512 * (qq + 1)],
                        otex[sub][0:64, :], ps_b[:])
            drain_unit_boundary()
            while units:
                for _ in units.pop(0):
                    pass
            for t in range(4 * qq, 4 * qq + 4):
                p3_unit(t)


def make_in_maps(x, w_qkv, b_qkv, w_out, b_out):
    x = np.asarray(x, dtype=np.float32)
    w_qkv = np.asarray(w_qkv, dtype=np.float32)
    b_qkv = np.asarray(b_qkv, dtype=np.float32)
    w_out = np.asarray(w_out, dtype=np.float32)

    wrr = w_qkv.reshape(D, 3, 8, HD)
    br = b_qkv.reshape(3, 8, HD)
    tri = np.tril(np.ones((128, 128), dtype=np.float32)).T  # tri[k,x]=k<=x

    in_maps = []
    for c in range(NCORES):
        b = c // 2
        h0 = 4 * (c % 2)
        xT = np.ascontiguousarray(x[b].T)                       # (512, 2048)
        wq = wrr[:, 0, h0:h0 + 4].reshape(D, 256)
        wk = wrr[:, 1, h0:h0 + 4].reshape(D, 256)
        wv = wrr[:, 2, h0:h0 + 4].reshape(D, 256)
        bq = br[0, h0:h0 + 4].reshape(256)
        bk = br[1, h0:h0 + 4].reshape(256)
        bv = br[2, h0:h0 + 4].reshape(256)
        wva = np.zeros((D, VWS), dtype=np.float32)
        vb = np.zeros((VWS,), dtype=np.float32)
        for j in range(HPC):
            wva[:, VW * j:VW * j + HD] = wv[:, HD * j:HD * (j + 1)]
            vb[VW * j:VW * j + HD] = bv[HD * j:HD * (j + 1)]
            vb[VW * j + HD] = 1.0
        wo = w_out.reshape(8, HD, D)[h0:h0 + 4].reshape(256, D)

        wpack = np.zeros((128, FTOT), dtype=np.float32)
        wpack[:, OFF_BQK + 0] = bq[0:128]
        wpack[:, OFF_BQK + 1] = bq[128:256]
        wpack[:, OFF_BQK + 2] = bk[0:128]
        wpack[:, OFF_BQK + 3] = bk[128:256]
        wpack[0, OFF_ONES64:OFF_ONES64 + 64] = 1.0
        wpack[:, OFF_TRI:OFF_TRI + 128] = tri
        wpack[:, OFF_TRI + 128:OFF_TRI + 256] = tri
        wpack[:, OFF_VB:OFF_VB + VWS] = vb[None, :]
        for dc in range(4):
            wpack[:, OFF_WQ + 256 * dc:OFF_WQ + 256 * (dc + 1)] = \
                wq[128 * dc:128 * (dc + 1)]
            wpack[:, OFF_WK + 256 * dc:OFF_WK + 256 * (dc + 1)] = \
                wk[128 * dc:128 * (dc + 1)]
            wpack[:, OFF_WVA + VWS * dc:OFF_WVA + VWS * (dc + 1)] = \
                wva[128 * dc:128 * (dc + 1)]
        for p in range(2):
            wpack[:, OFF_WO + D * p:OFF_WO + D * (p + 1)] = \
                wo[128 * p:128 * (p + 1)]
        for sc in range(4):
            for dc in range(4):
                a = OFF_XT + S * sc + 512 * dc
                wpack[:, a:a + 512] = \
                    xT[128 * dc:128 * (dc + 1), 512 * sc:512 * (sc + 1)]

        in_maps.append({"wpack": wpack.astype(ml_dtypes.bfloat16)})
    return in_maps


_NC_CACHE = None


def get_nc():
    global _NC_CACHE
    if _NC_CACHE is None:
        _NC_CACHE = build_nc()
    return _NC_CACHE


def run_cores(x, w_qkv, b_qkv, w_out, b_out, trace=False, trace_cores=None):
    nc = get_nc()
    in_maps = make_in_maps(x, w_qkv, b_qkv, w_out, b_out)
    br = run_bass_kernel_spmd(
        nc, in_maps, list(range(NCORES)),
        trace=trace, trace_cores=trace_cores)
    return br


def assemble(results, b_out):
    b_out = np.asarray(b_out, dtype=np.float32)
    out = np.empty((4, S, D), dtype=np.float32)
    for b in range(4):
        out[b] = results[2 * b]["out"] + results[2 * b + 1]["out"] + b_out
    return out


def kernel(x, w_qkv, b_qkv, w_out, b_out):
    br = run_cores(x, w_qkv, b_qkv, w_out, b_out, trace=False)
    return assemble(br.results, b_out)


# revision 27
# speedup vs baseline: 1.0773x; 1.0773x over previous
"""Causal multi-head attention kernel for 8 Trainium2 NeuronCores.

Problem: x(4,2048,512) -> qkv proj -> 8-head causal attention -> out proj.
Sharding: core c handles batch b=c//2, heads 4*(c%2)..4*(c%2)+3.
Each core returns a partial (2048,512) output (its 4 heads' contribution
through w_out); host sums the two cores of each batch and adds b_out.

The attention inner loop is ACT(exp)-bound (~1 elem/cycle/lane), so the
kernel software-pipelines every other stage under it:
  - input DMA split into consumption-ordered chunks (weights -> xT sc0 ->
    remaining xT) so the QKV projection starts ~5us in;
  - Q/K projections (per 512-seq chunk), V-aug projection (per 128-seq
    chunk) and the output projection (per 128-row chunk) are emitted as
    "filler" PE work interleaved into the attention kk-loop;
  - causal diag masking is a bf16 DVE multiply on the exp'd tile (PE does
    no mask matmuls), Q/K biases fold into the DVE PSUM->SBUF evacuation
    (per-partition tensor_scalar add), V bias + denominator ones-column
    fold into a host-precomputed (128,260) tile added during evacuation;
  - output chunks DMA to HBM as soon as each 128-row slice is projected.

Per-core device algorithm (bf16 matmuls, fp32 psum/softmax):
  P1  QT/KT stored (128=2heads, 2048) per pair; vaug holds [V|1] per
      128-seq chunk so the PV matmul also produces softmax denominators.
  P2  per head-pair: S_T = K Q^T (k on partitions, q free, two heads
      packed into PE row-groups 0/64 -> concurrent), exp via ACT
      (scale=1/8 folded, no max subtraction -- scores are O(7)), diag
      mask via DVE multiply, then out'/denom = [V|1]^T @ P~^T in PSUM.
  P3  denominators DMA-gathered to (16,64), one DVE reciprocal, DMA back
      to a row, partition-broadcast via a ones(1,64) matmul, normalize,
      project, stream out per 128-row chunk.
"""

import os
import sys

import numpy as np

if "/opt/trn_rl_repo" not in sys.path:
    sys.path.insert(0, "/opt/trn_rl_repo")

import ml_dtypes

import concourse.bass as bass
import concourse.mybir as mybir
import concourse.tile as tile
from concourse import bacc
from concourse.bass_utils import run_bass_kernel_spmd

F32 = mybir.dt.float32
BF16 = mybir.dt.bfloat16
AF = mybir.ActivationFunctionType

S = 2048
D = 512
HD = 64
HPC = 4          # heads per core
NCORES = 8
SCALE = 0.125    # 1/sqrt(64)
VW = HD + 1      # 65: V plus ones column
VWS = HPC * VW   # 260

# column offsets inside the packed bf16 (128, FTOT) input, ordered by
# consumption so the input DMA can stream in 3 contiguous chunks:
# [everything the prefix needs][xT sc1-3][wo]
OFF_BQK = 0                     # (128,4): bq_p0, bq_p1, bk_p0, bk_p1 cols
OFF_ONES64 = 4                  # row 0: ones(64)
OFF_TRI = 68                    # (128,256): tri|tri, tri[k,x]=1 if k<=x
OFF_VB = OFF_TRI + 256          # (128,260): [bv|1] per head, replicated
OFF_WQ = OFF_VB + VWS           # 4 tiles of (128, 256)
OFF_WK = OFF_WQ + 4 * 256
OFF_WVA = OFF_WK + 4 * 256      # 4 tiles of (128, 260)
OFF_XT = OFF_WVA + 4 * VWS      # 4 sc-chunks of (4 dc x (128, 512))
OFF_WO = OFF_XT + 4 * S         # 2 tiles of (128, 512)
FTOT = OFF_WO + 2 * D


def build_nc():
    nc = bacc.Bacc("TRN2", target_bir_lowering=False, debug=False)

    wpack = nc.dram_tensor("wpack", [128, FTOT], BF16,
                           kind="ExternalInput").ap()
    out = nc.dram_tensor("out", [S, D], F32, kind="ExternalOutput").ap()

    with tile.TileContext(nc) as tc:
        _build_kernel(tc, wpack, out)
    nc.compile()
    return nc


def _build_kernel(tc, wpack, out):
    nc = tc.nc
    from contextlib import ExitStack

    ctx = ExitStack()
    with ctx:
        pers = ctx.enter_context(tc.tile_pool(name="pers", bufs=1))
        spsum = ctx.enter_context(
            tc.tile_pool(name="spsum", bufs=2, space="PSUM"))   # scores
        opsum = ctx.enter_context(
            tc.tile_pool(name="opsum", bufs=1, space="PSUM"))   # PV accum
        upsum = ctx.enter_context(
            tc.tile_pool(name="upsum", bufs=2, space="PSUM"))   # p1/p3/bcast
        ptp = ctx.enter_context(tc.tile_pool(name="ptp", bufs=4))
        otexp = ctx.enter_context(tc.tile_pool(name="otexp", bufs=4))
        outp = ctx.enter_context(tc.tile_pool(name="outp", bufs=3))
        dnp = ctx.enter_context(tc.tile_pool(name="dnp", bufs=2))

        # ---------- input DMA, split in consumption order ----------
        wr = pers.tile([128, FTOT], BF16, tag="wr", name="wr")
        dma_splits = [
            (0, OFF_XT + S),                   # header + weights + xT sc0
            (OFF_XT + S, OFF_XT + 4 * S),      # xT sc1-3
            (OFF_WO, FTOT),                    # wo (first needed by P3)
        ]
        for a, b in dma_splits:
            nc.sync.dma_start(wr[:, a:b], wpack[:, a:b])

        def xt_sb(dc, sc):
            base = OFF_XT + S * sc + 512 * dc
            return wr[:, base:base + 512]

        def xt_128(st):
            sc, r = st // 4, st % 4
            base = OFF_XT + S * sc
            return [wr[:, base + 512 * dc + 128 * r:
                       base + 512 * dc + 128 * r + 128] for dc in range(4)]

        wq_sb = [wr[:, OFF_WQ + 256 * dc:OFF_WQ + 256 * (dc + 1)]
                 for dc in range(4)]
        wk_sb = [wr[:, OFF_WK + 256 * dc:OFF_WK + 256 * (dc + 1)]
                 for dc in range(4)]
        wva_sb = [wr[:, OFF_WVA + VWS * dc:OFF_WVA + VWS * (dc + 1)]
                  for dc in range(4)]
        wo_sb = [wr[:, OFF_WO + D * p:OFF_WO + D * (p + 1)]
                 for p in range(2)]
        vb_sb = wr[:, OFF_VB:OFF_VB + VWS]
        # per-partition bias columns for tensor_scalar must be fp32
        bcols = pers.tile([128, 4], F32, tag="bcols", name="bcols")
        nc.vector.tensor_copy(bcols[:], wr[:, OFF_BQK:OFF_BQK + 4])
        bq_col = [bcols[:, p:p + 1] for p in range(2)]
        bk_col = [bcols[:, 2 + p:3 + p] for p in range(2)]
        ones64 = wr[0:1, OFF_ONES64:OFF_ONES64 + 64]
        tri2 = wr[:, OFF_TRI:OFF_TRI + 256].rearrange(
            "p (b c) -> p b c", c=128)

        # ---------- persistent state ----------
        QT, KT, OTN = [], [], []
        for p in range(2):
            QT.append(pers.tile([128, S], BF16, tag=f"QT{p}", name=f"QT{p}"))
            KT.append(pers.tile([128, S], BF16, tag=f"KT{p}", name=f"KT{p}"))
            OTN.append(pers.tile([128, S], BF16, tag=f"OTN{p}",
                                 name=f"OTN{p}"))
        vaug = pers.tile([128, 16 * VWS], BF16, tag="vaug", name="vaug")

        # memset the scores psum buffers once (safety for first-use lanes)
        # and run a throwaway exp on one so the ACT table load (~2.7us)
        # happens during the input-DMA wait instead of at the first score
        warm = pers.tile([1, 8], BF16, tag="warm", name="warm")
        for i in range(2):
            ps_init = spsum.tile([128, 1024], F32, tag="ps_s", name="ps_i")
            nc.vector.memset(ps_init[:], 0.0)
            if i == 0:
                nc.scalar.activation(warm[:], ps_init[0:1, 0:8], AF.Exp,
                                     scale=SCALE)

        # ---------- work units ----------
        # p1 units are generators yielding after every matmul, so the main
        # loop can interleave filler PE work at single-matmul granularity
        # (a whole 4-matmul unit in one burst delays the next QK enough to
        # starve the exp stream)
        def p1a_unit(p, which, sc):
            """Project one 512-seq chunk of Q or K for pair p."""
            w_sb = wq_sb if which == "q" else wk_sb
            b_col = bq_col[p] if which == "q" else bk_col[p]
            dst = QT[p] if which == "q" else KT[p]
            ps = upsum.tile([128, 512], F32, tag="util", name="p1ps")
            for dc in range(4):
                nc.tensor.matmul(
                    ps[:],
                    w_sb[dc][:, 128 * p:128 * (p + 1)],
                    xt_sb(dc, sc),
                    start=(dc == 0), stop=(dc == 3))
                yield
            nc.vector.tensor_scalar_add(
                dst[:, 512 * sc:512 * (sc + 1)], ps[:], b_col)
            yield

        def p1b_unit(st):
            """Project one 128-seq chunk of [V|1] (all 4 heads)."""
            xbl = xt_128(st)
            ps = upsum.tile([128, 512], F32, tag="util", name="p1vps")
            for dc in range(4):
                nc.tensor.matmul(
                    ps[:, 0:VWS],
                    xbl[dc],
                    wva_sb[dc][:],
                    start=(dc == 0), stop=(dc == 3))
                yield
            nc.vector.tensor_add(vaug[:, VWS * st:VWS * (st + 1)],
                                 ps[:, 0:VWS], vb_sb)
            yield

        def run_unit(gen):
            for _ in gen:
                pass

        def p3_unit(t):
            """Project + stream out one 128-row output chunk."""
            ps = upsum.tile([128, 512], F32, tag="util", name="p3ps")
            for p in range(2):
                nc.tensor.matmul(
                    ps[:],
                    OTN[p][:, 128 * t:128 * (t + 1)],
                    wo_sb[p][:],
                    start=(p == 0), stop=(p == 1))
            osb = outp.tile([128, 512], F32, tag="osb", name="osb")
            nc.vector.tensor_copy(osb[:], ps[:])
            nc.sync.dma_start(out[128 * t:128 * (t + 1), :], osb[:])

        # ---------- prefix: minimum P1 to start pair 0's attention ----------
        for st in range(4):
            run_unit(p1b_unit(st))
        run_unit(p1a_unit(0, "q", 0))
        run_unit(p1a_unit(0, "k", 0))

        # ---------- main attention loop, fillers interleaved ----------
        for qq in range(4):
            units = []
            if qq == 0:
                # pair 1's sc0 projections must land before its kk loop
                units.append(p1a_unit(1, "q", 0))
                units.append(p1a_unit(1, "k", 0))
            if qq < 3:
                for p in range(2):
                    units.append(p1a_unit(p, "q", qq + 1))
                    units.append(p1a_unit(p, "k", qq + 1))
                for st in range(4 * qq + 4, 4 * qq + 8):
                    units.append(p1b_unit(st))
            # one flat stream of single-matmul filler steps for this qq.
            # state[0] is the in-flight unit: it must be drained to a unit
            # boundary before anything else allocates from the util ring
            # (an open accumulation group + 2 more allocs would wrap the
            # 2-buffer ring and clobber the accumulating psum tile)
            state = [None]
            nstep0 = len(units) * 5
            nstep = 0

            def step_filler():
                while True:
                    if state[0] is None:
                        if not units:
                            return False
                        state[0] = units.pop(0)
                    try:
                        next(state[0])
                        return True
                    except StopIteration:
                        state[0] = None

            def drain_unit_boundary():
                if state[0] is not None:
                    for _ in state[0]:
                        pass
                    state[0] = None

            nkk = 4 * qq + 4
            niter = 2 * nkk
            it = 0
            for p in range(2):
                ps_oo = []
                for sub in range(2):
                    t = opsum.tile([VW, 512], F32, tag=f"ps_o{sub}",
                                   name=f"ps_o{sub}")
                    ps_oo.append(t)
                for kk in range(nkk):
                    diag = (kk >= 4 * qq)
                    so = 128 * (kk - 4 * qq) if diag else 0
                    # the QK->exp->PV chain runs in a high-priority band so
                    # the scheduler never front-loads ready filler matmuls
                    # ahead of the next QK (which starves the exp stream)
                    with tc.high_priority(offset=1 << 20):
                        ps_s = spsum.tile([128, 1024], F32, tag="ps_s",
                                          name="ps_s")
                        for sub in range(2):
                            qrows = slice(64 * sub, 64 * sub + 64)
                            nc.tensor.matmul(
                                ps_s[:, 512 * sub + so:512 * (sub + 1)],
                                KT[p][qrows, 128 * kk:128 * (kk + 1)],
                                QT[p][qrows, 512 * qq + so:512 * (qq + 1)],
                                start=True, stop=True)
                        pt = ptp.tile([128, 1024], BF16, tag="pt", name="pt")
                        if so == 0:
                            nc.scalar.activation(pt[:], ps_s[:], AF.Exp,
                                                 scale=SCALE)
                        else:
                            pss3 = ps_s[:].rearrange(
                                "p (b c) -> p b c", c=512)
                            pt3 = pt[:].rearrange("p (b c) -> p b c", c=512)
                            nc.scalar.activation(pt3[:, :, so:],
                                                 pss3[:, :, so:],
                                                 AF.Exp, scale=SCALE)
                        if diag:
                            # zero the strictly-upper (k>q) part of the
                            # 128-wide diag block on DVE: one bf16 multiply
                            ptd = pt[:].rearrange(
                                "p (b c) -> p b c", c=512)[:, :, so:so + 128]
                            nc.vector.tensor_mul(ptd, ptd, tri2)
                        for sub in range(2):
                            h = 2 * p + sub
                            nc.tensor.matmul(
                                ps_oo[sub][:, so:512],
                                vaug[:, VWS * kk + VW * h:
                                     VWS * kk + VW * h + VW],
                                pt[:, 512 * sub + so:512 * (sub + 1)],
                                start=(kk == 0), stop=(kk == nkk - 1))
                    it += 1
                    # interleave filler PE work under the ACT-bound loop:
                    # after iteration `it`, nstep0*it/niter steps are due
                    while nstep < nstep0 * it // niter:
                        if not step_filler():
                            nstep = nstep0
                            break
                        nstep += 1
                # evacuate PV accum + gather denominators for this (p, qq).
                # the last qq's gathers ride the scalar HWDGE ring: ACT is
                # idle by then and the sync ring is busy streaming output
                drain_unit_boundary()
                dma_eng = nc.scalar if qq == 3 else nc.sync
                with tc.high_priority(offset=1 << 20):
                    dq = dnp.tile([16, 64], F32, tag="dq", name="dq")
                    otex = []
                    for sub in range(2):
                        ot = otexp.tile([VW, 512], F32, tag="otex",
                                        name="otex")
                        nc.vector.tensor_copy(ot[:], ps_oo[sub][:])
                        dma_eng.dma_start(dq[8 * sub:8 * sub + 8, :],
                                          ot[64:65, :])
                        otex.append(ot)
                    rq = dnp.tile([16, 64], BF16, tag="rq", name="rq")
                    with nc.allow_low_precision(reason="bf16 softmax recip"):
                        nc.vector.reciprocal(rq[:], dq[:])
                    rrow = dnp.tile([1, 1024], BF16, tag="rrow",
                                    name="rrow")
                    dma_eng.dma_start(rrow[:], rq[:])
                    for sub in range(2):
                        ps_b = upsum.tile([64, 512], F32, tag="util",
                                          name="ps_b")
                        nc.tensor.matmul(
                            ps_b[:],
                            ones64,
                            rrow[0:1, 512 * sub:512 * (sub + 1)],
                            start=True, stop=True)
                        nc.vector.tensor_mul(
                            OTN[p][64 * sub:64 * sub + 64,
                                   512 * qq:512 * (qq + 1)],
                            otex[sub][0:64, :], ps_b[:])
            drain_unit_boundary()
            while units:
                for _ in units.pop(0):
                    pass
            for t in range(4 * qq, 4 * qq + 4):
                p3_unit(t)


def make_in_maps(x, w_qkv, b_qkv, w_out, b_out):
    x = np.asarray(x, dtype=np.float32)
    w_qkv = np.asarray(w_qkv, dtype=np.float32)
    b_qkv = np.asarray(b_qkv, dtype=np.float32)
    w_out = np.asarray(w_out, dtype=np.float32)

    wrr = w_qkv.reshape(D, 3, 8, HD)
    br = b_qkv.reshape(3, 8, HD)
    tri = np.tril(np.ones((128, 128), dtype=np.float32)).T  # tri[k,x]=k<=x

    in_maps = []
    for c in range(NCORES):
        b = c // 2
        h0 = 4 * (c % 2)
        xT = np.ascontiguousarray(x[b].T)                       # (512, 2048)
        wq = wrr[:, 0, h0:h0 + 4].reshape(D, 256)
        wk = wrr[:, 1, h0:h0 + 4].reshape(D, 256)
        wv = wrr[:, 2, h0:h0 + 4].reshape(D, 256)
        bq = br[0, h0:h0 + 4].reshape(256)
        bk = br[1, h0:h0 + 4].reshape(256)
        bv = br[2, h0:h0 + 4].reshape(256)
        wva = np.zeros((D, VWS), dtype=np.float32)
        vb = np.zeros((VWS,), dtype=np.float32)
        for j in range(HPC):
            wva[:, VW * j:VW * j + HD] = wv[:, HD * j:HD * (j + 1)]
            vb[VW * j:VW * j + HD] = bv[HD * j:HD * (j + 1)]
            vb[VW * j + HD] = 1.0
        wo = w_out.reshape(8, HD, D)[h0:h0 + 4].reshape(256, D)

        wpack = np.zeros((128, FTOT), dtype=np.float32)
        wpack[:, OFF_BQK + 0] = bq[0:128]
        wpack[:, OFF_BQK + 1] = bq[128:256]
        wpack[:, OFF_BQK + 2] = bk[0:128]
        wpack[:, OFF_BQK + 3] = bk[128:256]
        wpack[0, OFF_ONES64:OFF_ONES64 + 64] = 1.0
        wpack[:, OFF_TRI:OFF_TRI + 128] = tri
        wpack[:, OFF_TRI + 128:OFF_TRI + 256] = tri
        wpack[:, OFF_VB:OFF_VB + VWS] = vb[None, :]
        for dc in range(4):
            wpack[:, OFF_WQ + 256 * dc:OFF_WQ + 256 * (dc + 1)] = \
                wq[128 * dc:128 * (dc + 1)]
            wpack[:, OFF_WK + 256 * dc:OFF_WK + 256 * (dc + 1)] = \
                wk[128 * dc:128 * (dc + 1)]
            wpack[:, OFF_WVA + VWS * dc:OFF_WVA + VWS * (dc + 1)] = \
                wva[128 * dc:128 * (dc + 1)]
        for p in range(2):
            wpack[:, OFF_WO + D * p:OFF_WO + D * (p + 1)] = \
                wo[128 * p:128 * (p + 1)]
        for sc in range(4):
            for dc in range(4):
                a = OFF_XT + S * sc + 512 * dc
                wpack[:, a:a + 512] = \
                    xT[128 * dc:128 * (dc + 1), 512 * sc:512 * (sc + 1)]

        in_maps.append({"wpack": wpack.astype(ml_dtypes.bfloat16)})
    return in_maps


_NC_CACHE = None


def get_nc():
    global _NC_CACHE
    if _NC_CACHE is None:
        _NC_CACHE = build_nc()
    return _NC_CACHE


def run_cores(x, w_qkv, b_qkv, w_out, b_out, trace=False, trace_cores=None):
    nc = get_nc()
    in_maps = make_in_maps(x, w_qkv, b_qkv, w_out, b_out)
    br = run_bass_kernel_spmd(
        nc, in_maps, list(range(NCORES)),
        trace=trace, trace_cores=trace_cores)
    return br


def assemble(results, b_out):
    b_out = np.asarray(b_out, dtype=np.float32)
    out = np.empty((4, S, D), dtype=np.float32)
    for b in range(4):
        out[b] = results[2 * b]["out"] + results[2 * b + 1]["out"] + b_out
    return out


def kernel(x, w_qkv, b_qkv, w_out, b_out):
    br = run_cores(x, w_qkv, b_qkv, w_out, b_out, trace=False)
    return assemble(br.results, b_out)
